# revision 56
# baseline (speedup 1.0000x reference)
"""Trainium2 Bass kernel for nn_AutoDecoder (moe_routing).

Reference computation (per full input):
  x: [S=3072, B=32, C=512]; rows s%3==1 are "brick" tokens, s%3==2 are
  "combined" tokens (s%3==0 PAD rows are dead). For each (timestep, batch)
  pair:
    brick:  logits[0:80]    = x_brick @ [Ws|Wc]            (+ biases)
    comb:   h = relu(relu(x_comb @ W1 + b1) @ W2 + b2)
            logits[80:1000] = h @ Wh + bh
  out: [TS=1024, B=32, A=1000]

Strategy: data-parallel over batch (4 batch entries per core, 8 cores),
weights replicated. The host pre-routes the brick/combined tokens,
casts to fp16 and pre-transposes each name's activations to
feature-major [C, ntok] (all layout work is host-side prep, like the
weight reshapes) so the device runs zero transposes: the 2-layer MLP
streams the feature-major tokens directly against fp16 weights with
fp32 PSUM accumulation, and the head matmuls use feature-major
activations as stationary operands to produce token-major logits,
biased on DVE and written back as fp16 with fully contiguous DMA (the
host upcasts the gathered output to fp32).

Per block of 512 tokens the PE runs L1 (16 matmuls), then the previous
block's heads (48 matmuls — fills the ACT-latency bubble before L2),
then L2, giving near-continuous PE streaming; fp16 operands keep
11-bit multiply precision at 1 column/cycle, accumulation is fp32.
"""
import sys

if "/opt/trn_rl_repo" not in sys.path:
    sys.path.append("/opt/trn_rl_repo")

import numpy as np

import concourse.bass as bass
from concourse import bacc
import concourse.mybir as mybir
import concourse.tile as tile
from concourse.bass import ts
from concourse.bass_utils import run_bass_kernel_spmd

F32 = mybir.dt.float32
F16 = mybir.dt.float16
RELU = mybir.ActivationFunctionType.Relu

# problem dims (hardcoded; kernel.py must be self-contained)
S, B, C = 3072, 32, 512
TS_ = S // 3                    # 1024 timesteps
NUM_SHAPES, NUM_COLORS, N_COMBINED = 64, 16, 920
NBRICK = NUM_SHAPES + NUM_COLORS  # 80
A = NBRICK + N_COMBINED           # 1000
NCORES = 8
BL = B // NCORES                  # 4 batch entries per core
TT = 128                          # tokens per tok-tile
TPB = TT // BL                    # 32 timesteps per tok-tile
NTOK = TS_ * BL                   # 4096 tokens per name per core
KC = C // 128                     # 4 contraction chunks
# block schedule (tok-tiles per block): small blocks at the ends for fast
# pipeline ramp/drain. Shared by the device program and the host layout.
SCHED = [1, 1, 2, 4, 4, 4, 4, 4, 4, 2, 1, 1]

_BUILD_CACHE = {}


def _build():
    if "nc" in _BUILD_CACHE:
        return _BUILD_CACHE["nc"]
    nc = bacc.Bacc("TRN2", target_bir_lowering=False, debug=False)

    # feature-major activations, block-major layout: for each block the
    # host stores [128, KC, W] contiguously so every x DMA moves one
    # (KC*W*2)-byte contiguous run per partition.
    xt_d = nc.declare_dram_parameter("xt", [2, KC * 128 * NTOK], F16, isOutput=False)
    w1_d = nc.declare_dram_parameter("w1", [128, KC, C], F16, isOutput=False)
    w2_d = nc.declare_dram_parameter("w2", [128, KC, C], F16, isOutput=False)
    wh_d = nc.declare_dram_parameter("wh", [128, KC, N_COMBINED], F16, isOutput=False)
    wsc_d = nc.declare_dram_parameter("wsc", [128, KC, NBRICK], F16, isOutput=False)
    b12_d = nc.declare_dram_parameter("b12t", [128, 2 * KC], F32, isOutput=False)
    bA_d = nc.declare_dram_parameter("biasA", [128, A], F16, isOutput=False)
    out_d = nc.declare_dram_parameter("out", [TS_, BL, A], F16, isOutput=True)

    xv = xt_d[:]

    with tile.TileContext(nc) as tc:
        with (
            tc.tile_pool(name="const", bufs=1) as const,
            tc.tile_pool(name="xt", bufs=3) as xt_p,
            tc.tile_pool(name="h", bufs=2) as h_p,
            tc.tile_pool(name="osb", bufs=6) as o_p,
            tc.tile_pool(name="psh", bufs=4, space=bass.MemorySpace.PSUM) as ps_h,
            tc.tile_pool(name="psc", bufs=2, space=bass.MemorySpace.PSUM) as ps_c,
        ):
            # ---- HAM warmup: ~4us of dummy matmuls at t=0 (on a memset
            # scratch, no DMA dependency) so the PE clock gate is already
            # released when the real work arrives.
            warm_src = const.tile([128, 128], F16, tag="warm")
            nc.vector.memset(warm_src[:], 0.0)
            warm = ps_h.tile([128, 512], F32, tag="hps")
            for _ in range(46):
                nc.tensor.matmul(warm[:, 0:128], warm_src[:], warm_src[:])
            # pre-fire the one-time ACT activation-table load so the first
            # real relu doesn't pay ~1.3us for it
            warm_act = const.tile([128, 1], F32, tag="warmact")
            nc.scalar.activation(warm_act[0:1, 0:1], warm_src[0:1, 0:1], RELU)
            # consts are loaded with ONE DMA each — few enough early DMAs
            # that the ~19-deep semaphore pool never wraps at the head (a
            # wrap serializes unrelated queues behind big x transfers).
            # Order on sync = need order: w1 (L1@~12us), b12 (ACT@~13),
            # w2 (L2@~13.5), wh/wsc (finals(0)@~15).
            # halves: consumers unblock on half-arrival instead of waiting
            # for the full 0.5MB transfer
            w1_sb = const.tile([128, KC, C], F16, tag="w1")
            nc.sync.dma_start(w1_sb[:, 0:2, :], w1_d[:, 0:2, :])
            nc.sync.dma_start(w1_sb[:, 2:KC, :], w1_d[:, 2:KC, :])
            b12_sb = const.tile([128, 2 * KC], F32, tag="b12")
            nc.sync.dma_start(b12_sb[:], b12_d[:, :])
            w2_sb = const.tile([128, KC, C], F16, tag="w2")
            nc.sync.dma_start(w2_sb[:, 0:2, :], w2_d[:, 0:2, :])
            nc.sync.dma_start(w2_sb[:, 2:KC, :], w2_d[:, 2:KC, :])

            def load_deferred_consts():
                # wh (0.94MB) rides the otherwise-idle scalar queue so it
                # doesn't queue behind w1/w2 on sync — needed by finals(0)
                wh_sb = const.tile([128, KC, N_COMBINED], F16, tag="wh")
                nc.scalar.dma_start(wh_sb[:, 0:2, :], wh_d[:, 0:2, :])
                nc.scalar.dma_start(wh_sb[:, 2:KC, :], wh_d[:, 2:KC, :])
                wsc_sb = const.tile([128, KC, NBRICK], F16, tag="wsc")
                nc.sync.dma_start(wsc_sb[:], wsc_d[:, :, :])
                bA_sb = const.tile([128, A], F16, tag="biasA")
                nc.scalar.dma_start(bA_sb[:], bA_d[:, :])
                return wh_sb, wsc_sb, bA_sb

            # ---- main loop over blocks of tok-tiles (128 tokens each) ----
            # ramp-up/ramp-down schedule: small blocks first (fast pipeline
            # fill, early HAM release) and last (short drain tail).
            # Heads ("finals") for block i are emitted during block i+1 —
            # between L1 and L2 — so the PE never waits on ACT's h1 drain
            # and streams continuously in steady state.
            def finals(pb):
                for t in range(pb["nt"]):
                    pco = ps_c.tile([128, 1024], F32, tag="combo")
                    for k in range(KC):
                        lhs = pb["h2"][k][:, ts(t, 128)]
                        nc.tensor.matmul(
                            pco[:, 0:512],
                            lhs,
                            wh_sb[:, k, 0:512],
                            start=(k == 0),
                            stop=(k == KC - 1),
                        )
                        nc.tensor.matmul(
                            pco[:, 512:N_COMBINED],
                            lhs,
                            wh_sb[:, k, 512:N_COMBINED],
                            start=(k == 0),
                            stop=(k == KC - 1),
                        )
                    for k in range(KC):
                        nc.tensor.matmul(
                            pco[:, N_COMBINED:A],
                            pb["xb"][:, k, ts(t, 128)],
                            wsc_sb[:, k, :],
                            start=(k == 0),
                            stop=(k == KC - 1),
                        )
                    ot = o_p.tile([128, A], F16, tag="osb")
                    orow = out_d[pb["t0"] + t * TPB : pb["t0"] + (t + 1) * TPB, :, :]
                    if pb.get("last"):
                        # drain-tail blocks: split adds/DMA so the writeback
                        # overlaps the remaining head matmuls
                        nc.vector.tensor_add(
                            ot[:, NBRICK : NBRICK + 512],
                            pco[:, 0:512],
                            bA_sb[:, NBRICK : NBRICK + 512],
                        )
                        nc.gpsimd.dma_start(
                            orow[:, :, NBRICK : NBRICK + 512],
                            ot[:, NBRICK : NBRICK + 512],
                        )
                        nc.vector.tensor_add(
                            ot[:, NBRICK + 512 : A],
                            pco[:, 512:N_COMBINED],
                            bA_sb[:, NBRICK + 512 : A],
                        )
                        nc.vector.tensor_add(
                            ot[:, 0:NBRICK], pco[:, N_COMBINED:A], bA_sb[:, 0:NBRICK]
                        )
                        nc.gpsimd.dma_start(
                            orow[:, :, NBRICK + 512 : A], ot[:, NBRICK + 512 : A]
                        )
                        nc.sync.dma_start(orow[:, :, 0:NBRICK], ot[:, 0:NBRICK])
                    else:
                        nc.vector.tensor_add(
                            ot[:, NBRICK:A], pco[:, 0:N_COMBINED], bA_sb[:, NBRICK:A]
                        )
                        nc.vector.tensor_add(
                            ot[:, 0:NBRICK], pco[:, N_COMBINED:A], bA_sb[:, 0:NBRICK]
                        )
                        nc.sync.dma_start(orow[:], ot[:])

            assert sum(SCHED) == TS_ // TPB
            ti0 = 0
            off = 0  # element offset into each name's block-major x image
            pending = None
            for nt in SCHED:
                t0 = ti0 * TPB
                W_ = nt * TT  # tokens per name in this block
                blk_elems = 128 * KC * W_
                # feature-major fp16 loads, one DMA per name pulling the
                # block's contiguous [128, KC, W] image; comb (ni=1) here:
                # the MLP needs it right away. The brick tile is only read
                # by the finals during the NEXT block, so its load is issued
                # after L1 below — at the head this keeps the first blocks'
                # critical DMA bytes minimal.
                xc_t = xt_p.tile([128, KC, W_], F16, tag="xtc")
                nc.gpsimd.dma_start(
                    xc_t[:],
                    xv[1, off : off + blk_elems].rearrange(
                        "(p k w) -> p k w", p=128, k=KC
                    ),
                )
                if ti0 == 0:
                    wh_sb, wsc_sb, bA_sb = load_deferred_consts()

                # comb MLP layer 1: h1T[m] = relu(W1[:,m-chunk].T @ xT + b1)
                h1 = []
                for m in range(KC):
                    ph = ps_h.tile([128, W_], F32, tag="hps")
                    for k in range(KC):
                        nc.tensor.matmul(
                            ph[:],
                            w1_sb[:, k, ts(m, 128)],
                            xc_t[:, k, :],
                            start=(k == 0),
                            stop=(k == KC - 1),
                        )
                    hs = h_p.tile([128, W_], F16, tag=f"h1_{m}")
                    nc.scalar.activation(
                        hs[:], ph[:], RELU, bias=b12_sb[:, m : m + 1], scale=1.0
                    )
                    h1.append(hs)

                # scalar queue: spreads input bandwidth across a 3rd DMA
                # queue (ACT has plenty of slack for the issue cost)
                xb_t = xt_p.tile([128, KC, W_], F16, tag="xtb")
                nc.scalar.dma_start(
                    xb_t[:],
                    xv[0, off : off + blk_elems].rearrange(
                        "(p k w) -> p k w", p=128, k=KC
                    ),
                )

                # previous block's heads here: PE stays busy while ACT
                # finishes draining h1
                if pending is not None:
                    finals(pending)

                # layer 2
                h2 = []
                for m in range(KC):
                    ph = ps_h.tile([128, W_], F32, tag="hps")
                    for k in range(KC):
                        nc.tensor.matmul(
                            ph[:],
                            w2_sb[:, k, ts(m, 128)],
                            h1[k][:],
                            start=(k == 0),
                            stop=(k == KC - 1),
                        )
                    hs = h_p.tile([128, W_], F16, tag=f"h2_{m}")
                    nc.scalar.activation(
                        hs[:], ph[:], RELU, bias=b12_sb[:, KC + m : KC + m + 1], scale=1.0
                    )
                    h2.append(hs)

                # heads read h2 feature-major (comb) and xb_t (brick)
                pending = {"h2": h2, "xb": xb_t, "t0": t0, "nt": nt,
                           "last": ti0 >= 30}
                ti0 += nt
                off += blk_elems
            finals(pending)

    nc.compile()
    _BUILD_CACHE["nc"] = nc
    return nc


def _prepare_inputs(inputs):
    """Host-side prep: validate/normalize routing, shard over batch,
    pre-transpose activations to feature-major fp16, replicate weights.
    Returns in_maps for the 8 cores."""
    x = np.asarray(inputs["x"], dtype=np.float32)
    readout_x = np.asarray(inputs["readout_x"], dtype=np.int32)
    W1 = np.asarray(inputs["W1"], dtype=np.float32)
    W2 = np.asarray(inputs["W2"], dtype=np.float32)
    Wh = np.asarray(inputs["Wh"], dtype=np.float32)
    Ws = np.asarray(inputs["Ws"], dtype=np.float32)
    Wc = np.asarray(inputs["Wc"], dtype=np.float32)
    b1 = np.asarray(inputs["b1"], dtype=np.float32)
    b2 = np.asarray(inputs["b2"], dtype=np.float32)
    bh = np.asarray(inputs["bh"], dtype=np.float32)
    bs = np.asarray(inputs["bs"], dtype=np.float32)
    bc = np.asarray(inputs["bc"], dtype=np.float32)

    # The kernel hardcodes the cyclic PAD/brick/comb routing. If the actual
    # readout pattern differs, permute x on the host so the device sees the
    # canonical layout (mirrors jnp.nonzero(..., size=ntok) semantics).
    ntok = TS_ * B
    rf = readout_x.reshape(-1)
    canonical = np.array_equal(
        readout_x, np.broadcast_to((np.arange(S, dtype=np.int32) % 3)[:, None], (S, B))
    )
    if not canonical:
        xf = x.reshape(S * B, C)
        xc = np.zeros_like(x).reshape(S * B, C)
        for name_idx in (1, 2):
            idx = np.nonzero(rf == name_idx)[0]
            if idx.shape[0] < ntok:
                idx = np.pad(idx, (0, ntok - idx.shape[0]))
            else:
                idx = idx[:ntok]
            tgt = (3 * (np.arange(ntok) // B) + name_idx) * B + (np.arange(ntok) % B)
            xc[tgt] = xf[idx]
        x = xc.reshape(S, B, C)

    xb16 = x[1::3].astype(np.float16)  # [TS_, B, C]
    xc16 = x[2::3].astype(np.float16)

    def _wlayout(w):
        # [C, N] -> [128, KC, N]: row c_in = k*128 + p goes to [p, k, :],
        # making each SBUF weight load one contiguous run per partition
        return np.ascontiguousarray(
            w.astype(np.float16).reshape(KC, 128, -1).transpose(1, 0, 2)
        )

    Wsc = _wlayout(np.concatenate([Ws, Wc], axis=1))
    W1h = _wlayout(W1)
    W2h = _wlayout(W2)
    Whh = _wlayout(Wh)
    b12t = np.ascontiguousarray(
        np.concatenate([b1.reshape(KC, 128).T, b2.reshape(KC, 128).T], axis=1)
    )
    biasA = np.concatenate([bs, bc, bh]).astype(np.float16)
    biasA_b = np.ascontiguousarray(np.broadcast_to(biasA, (128, A)))

    in_maps = []
    for c in range(NCORES):
        xt = np.empty((2, KC * 128 * NTOK), dtype=np.float16)
        for ni, src in ((0, xb16), (1, xc16)):
            sl = src[:, c * BL : (c + 1) * BL, :].reshape(NTOK, C)
            xT = sl.T.reshape(KC, 128, NTOK)  # [k, p, tok]
            parts = []
            c0 = 0
            for nt in SCHED:
                W_ = nt * TT
                # block-major [p, k, w] contiguous image
                parts.append(
                    np.ascontiguousarray(
                        xT[:, :, c0 : c0 + W_].transpose(1, 0, 2)
                    ).reshape(-1)
                )
                c0 += W_
            xt[ni] = np.concatenate(parts)
        in_maps.append(
            {
                "xt": xt,
                "w1": W1h,
                "w2": W2h,
                "wh": Whh,
                "wsc": Wsc,
                "b12t": b12t,
                "biasA": biasA_b,
            }
        )
    return in_maps


def _run(inputs, trace=False, trace_kwargs=None):
    nc = _build()
    in_maps = _prepare_inputs(inputs)
    res = run_bass_kernel_spmd(
        nc,
        in_maps,
        list(range(NCORES)),
        trace=trace,
        **(trace_kwargs or {}),
    )
    out = np.empty((TS_, B, A), dtype=np.float32)
    for c in range(NCORES):
        out[:, c * BL : (c + 1) * BL, :] = res.results[c]["out"].astype(np.float32)
    return out, res


def kernel(**inputs) -> np.ndarray:
    out, _ = _run(inputs, trace=False)
    return out


if __name__ == "__main__":
    nc = _build()
    print("built OK")
